# revision 30
# baseline (speedup 1.0000x reference)
"""Trainium2 Bass kernel for nn_CvxDifflayer (batched PDHG LP solver).

Math (per batch row b, 300 iterations):
    u_{k+1} = clip(u_k - tau*(q + y_k @ K), 0, 1)
    P_k     = sigma*(u_{k+1} @ K.T) - sigma*h
    y_{k+1} = relu(y_k + 2*P_k - P_{k-1})   [track YP = y_k - P_{k-1}]
    out z   = u_300[:, V:]  reshaped (12, 12)

Device scheme (per 64-batch shard, fold-2 on 128 partitions):
    partition p = 64h + b; fold half h=0 -> features 0:578, h=1 -> 578:1156
    column split at 512: A-region (cols 0:512) + B-region (cols 512:578)
    G psum = TQ (DMA preload) + y @ tauK   (MM1, bf16 inputs, fp32 accum;
             K's zero block for (y1, node-cols) is skipped; -I block included)
    u = clip(U - G)                        (DVE, chunked 4x128 + 66)
    u feat-major via PE transposes + ACT copies (cast bf16)
    P psum = ones-row*(-sigma*h) + u @ sigK.T  (MM2 over all 1156 feats)
    T3 = 2P + YP; y = relu(T3); YP = y - P (DVE)
    y feat-major via PE transposes + ACT copy (cast bf16)
    Filler matmuls keep the PE busy so its clock ramps 1.2 -> 2.4 GHz.
"""

import sys

for _p in ("/opt/trn_rl_repo", "/opt/pypackages"):
    if _p not in sys.path:
        sys.path.insert(0, _p)

import numpy as np

N_GRID = 12
N = 144          # nodes
V = 1012         # directed edges
F = V + N        # 1156 primal vars
YR = 288         # dual vars
B = 512
BS = 64          # batch per core
N_CORES = 8
ITERS = 300
FOLD = 578       # features per fold half
FA = 512         # A-region columns
FB = FOLD - FA   # 66

# bf16 const slab layout
C_KB = 0                     # 3 blocks of tauK rows, 1156 wide each
C_KS = 3 * F                 # 10 blocks of sigK.T rows, 288 wide each
C_SH = C_KS + 10 * YR        # -sigma*h row (row 0)
CBW = C_SH + YR

# PE filler counts (tuned against the trace)
ND1 = 0    # after MM1, while DVE starts the u update
NDC = 0    # between MM2 chunk groups
ND2 = 0    # while DVE does T3/relu
ND3 = 0    # while ACT copies YFM

RW = (128, 128, 32)          # y contraction chunk rows


def _build_constants(A, A_pos, b):
    K = np.zeros((YR, F), np.float32)
    K[:N, :V] = A
    K[N:, :V] = A_pos
    K[N:, V:] = -np.eye(N, dtype=np.float32)
    h = np.concatenate([b.astype(np.float32), np.zeros(N, np.float32)])
    Kn = np.float32(np.sqrt(np.abs(K).sum(0).max() * np.abs(K).sum(1).max()))
    tau = np.float32(0.9) / Kn
    return K, h, tau


def _host_consts(K, h, tau):
    """bf16 const slab shared by all cores."""
    import ml_dtypes
    sigma = tau
    tauK = tau * K
    cb = np.zeros((128, CBW), np.float32)
    for ci in range(3):
        r0 = 128 * ci
        rw = RW[ci]
        cb[:rw, C_KB + F * ci:C_KB + F * ci + F] = tauK[r0:r0 + rw, :]
    # MM2 blocks m = 2t + h: feats 578h + 128t (t<4, 128 rows) or 578h+512
    # (t=4, 66 rows); rows = sigma*K.T[feat] = sigma*K[:, feat].T
    for m in range(10):
        t, hh = m // 2, m % 2
        f0 = FOLD * hh + (128 * t if t < 4 else FA)
        cw = 128 if t < 4 else FB
        cb[:cw, C_KS + YR * m:C_KS + YR * m + YR] = sigma * K[:, f0:f0 + cw].T
    cb[0, C_SH:C_SH + YR] = -sigma * h
    return cb.astype(ml_dtypes.bfloat16)


def _per_core_tq(w_shard, tau):
    """TQ image (128, 578): tau*q folded; q nonzero only on node feats."""
    tq = np.zeros((128, FOLD), np.float32)
    # node feats 1012:1156 live on fold half h=1: cols 434:512 (UA3) + 0:66 (UB)
    tq[64:128, 434:512] = tau * w_shard[:, 0:78]
    tq[64:128, 512:578] = tau * w_shard[:, 78:144]
    return tq


def _build_bass():
    from concourse import bass, mybir
    from concourse.tile import TileContext
    from concourse.tile_rust import add_dep_helper
    from concourse.mybir import AluOpType as op

    f32 = mybir.dt.float32
    f32r = mybir.dt.float32r
    bf16 = mybir.dt.bfloat16

    nc = bass.Bass()
    d_cb = nc.dram_tensor("cb", (128, CBW), bf16, kind="ExternalInput")
    d_tq = nc.dram_tensor("tq", (128, FOLD), f32, kind="ExternalInput")
    d_cf = nc.dram_tensor("cf", (64, YR), f32, kind="ExternalInput")
    d_z = nc.dram_tensor("z", (64, N), f32r, kind="ExternalOutput")

    with TileContext(nc) as tc:
        with (
            tc.tile_pool(name="sb", bufs=1) as sp,
            tc.tile_pool(name="psGA0", bufs=1, space="PSUM") as pGA0,
            tc.tile_pool(name="psGB0", bufs=1, space="PSUM") as pGB0,
            tc.tile_pool(name="psTP", bufs=1, space="PSUM") as pTP,
            tc.tile_pool(name="psTX", bufs=1, space="PSUM") as pTX,
            tc.tile_pool(name="psP", bufs=1, space="PSUM") as pP,
        ):
            CB = sp.tile([128, CBW], bf16)
            UA = [sp.tile([128, 128], f32r, name=f"ua{c}", tag=f"ua{c}")
                  for c in range(4)]
            UB = sp.tile([128, FB], f32r)
            TMP = sp.tile([128, 128], f32)
            TMPB = sp.tile([128, FB], f32)
            YBM = sp.tile([64, YR], f32r)
            T3 = sp.tile([64, YR], f32)
            YP = sp.tile([64, YR], f32)
            YFM = sp.tile([128, 192], bf16)
            UFM = [sp.tile([128, 128], bf16, name=f"ufm{c}", tag=f"ufm{c}")
                   for c in range(4)]
            UFMB = sp.tile([128, 128], bf16)
            ONES = sp.tile([1, 64], bf16)
            IDENT = sp.tile([128, 128], f32)
            IDENTR = sp.tile([128, 128], f32r)
            SCRD = sp.tile([32, 8], f32)
            SCRA = sp.tile([32, 8], f32)
            SCRP = sp.tile([32, 8], f32)
            SCRD2 = sp.tile([32, 8], f32)
            TQF = sp.tile([128, FOLD], f32)
            PRES = [sp.tile([128, 128], f32, name=f"pres{c}", tag=f"pres{c}")
                    for c in range(4)]
            PRESB = sp.tile([128, FB], f32)

            GA = pGA0.tile([128, 512], f32, name="ga0")
            GB = pGB0.tile([128, FB], f32, name="gb0")
            TP = [pTP.tile([128, 128], f32r, name=f"tp{c}", tag=f"tp{c}")
                  for c in range(4)]
            TXX = pTX.tile([128, 320], f32r)
            P = pP.tile([128, YR], f32)

            dma_cb = nc.sync.dma_start(CB[:, :], d_cb[:, :])
            dma_cf = nc.sync.dma_start(YP[:, :], d_cf[:, :])
            dma_tq = nc.sync.dma_start(TQF[:, :], d_tq[:, :])

            prev = {}

            def chain(eng, inst, *sync_deps):
                for d in sync_deps:
                    add_dep_helper(inst.ins, d.ins, True, "warm")
                if eng in prev:
                    add_dep_helper(inst.ins, prev[eng].ins, False, "order")
                prev[eng] = inst
                return inst

            def pe(inst, *d):
                return chain("pe", inst, *d)

            def dve(inst, *d):
                return chain("dve", inst, *d)

            def act(inst, *d):
                return chain("act", inst, *d)

            def pool(inst, *d):
                return chain("pool", inst, *d)

            pool_insts = [
                pool(nc.gpsimd.memset(IDENT[:, :], 0.0)),
                pool(nc.gpsimd.affine_select(
                    out=IDENT[:, :], in_=IDENT[:, :],
                    compare_op=mybir.AluOpType.not_equal, fill=1.0, base=0,
                    pattern=[[-1, 128]], channel_multiplier=1)),
                pool(nc.gpsimd.tensor_copy(IDENTR[:, :], IDENT[:, :])),
            ]
            # PRES_{-1} = -TQ  (u_0 = 0)
            pool(nc.gpsimd.tensor_scalar_mul(PRES[0][:, :],
                                             TQF[:, 0:128], -1.0), dma_tq)
            for c in range(1, 4):
                pool(nc.gpsimd.tensor_scalar_mul(
                    PRES[c][:, :], TQF[:, 128 * c:128 * c + 128], -1.0))
            pool(nc.gpsimd.tensor_scalar_mul(PRESB[:, :], TQF[:, FA:FOLD],
                                             -1.0))
            dve(nc.vector.memset(YFM[:, :], 0.0))
            dve(nc.vector.memset(ONES[:, :], 1.0))
            dve(nc.vector.tensor_copy(SCRD[0:32, 0:4], YP[0:32, 0:4]), dma_cf)
            dve(nc.vector.tensor_copy(SCRD[0:32, 4:8], SCRD[0:32, 0:4]),
                prev["dve"])
            # ACT init absorber: cover DVE init writes (YFM memset WAW)
            act(nc.scalar.copy(SCRA[0:32, 0:4], SCRA[0:32, 4:8]),
                prev["dve"])

            # PE warmups: absorb one foreign sem each
            pe(nc.tensor.matmul(P[64:128, 0:64], CB[0:32, 0:64],
                                CB[0:32, 0:64], start=True, stop=True,
                                skip_group_check=True),
               dma_cb)
            pe(nc.tensor.transpose(TP[0][:, :], IDENTR[:, :],
                                   IDENTR[:, :]),
               *pool_insts)
            pe(nc.tensor.matmul(P[64:128, 0:64], YFM[0:32, 0:64],
                                CB[0:32, 0:64], start=True, stop=True,
                                skip_group_check=True))

            def dum(free=YR):
                pe(nc.tensor.matmul(P[64:128, 0:free], CB[0:128, 0:64],
                                    CB[0:128, C_KB:C_KB + free],
                                    start=True, stop=True,
                                    skip_group_check=True))

            def mm1(g, ci, h, f_lo, f_hi, psum_lo, start=False, stop=False):
                rw = RW[ci]
                pe(nc.tensor.matmul(
                    g[64 * h:64 * h + 64, psum_lo:psum_lo + (f_hi - f_lo)],
                    YFM[0:rw, 64 * ci:64 * ci + 64],
                    CB[0:rw, C_KB + F * ci + f_lo:C_KB + F * ci + f_hi],
                    start=start, stop=stop))

            def mm2(m, stop=False):
                t, hh = m // 2, m % 2
                cw = 128 if t < 4 else FB
                if t < 4:
                    lhs = UFM[t][0:cw, 64 * hh:64 * hh + 64]
                else:
                    lhs = UFMB[0:cw, 64 * hh:64 * hh + 64]
                pe(nc.tensor.matmul(
                    P[0:64, :], lhs,
                    CB[0:cw, C_KS + YR * m:C_KS + YR * m + YR],
                    start=False, stop=stop, skip_group_check=True))

            # pre-ramp the PE clock: ~4us of wait-free matmuls
            for _ in range(35):
                pe(nc.tensor.matmul(P[64:128, 0:YR], CB[0:128, 0:64],
                                    CB[0:128, C_KB:C_KB + YR],
                                    start=True, stop=True,
                                    skip_group_check=True))

            for it in range(ITERS):
                last = it == ITERS - 1
                ga, gb = GA, GB

                # DVE absorber: cover pool's PRES writes from last iteration
                dve(nc.vector.tensor_copy(SCRD2[0:32, 0:2], SCRD2[0:32, 2:4]),
                    prev["pool"])
                # pool self-sync spacer: cover pool-sem deps (PRES WAW)
                spp = nc.gpsimd.tensor_copy(SCRP[0:32, 0:4], SCRP[0:32, 4:8])
                add_dep_helper(spp.ins, prev["pool"].ins, True, "poolsync")
                chain("pool", spp)
                # ACT self-sync spacer: cover ACT-sem deps up to copyY(k-1)
                if it > 0:
                    sp1 = nc.scalar.copy(SCRA[0:32, 0:4], SCRA[0:32, 4:8])
                    add_dep_helper(sp1.ins, prev["act"].ins, True, "actsync")
                    chain("act", sp1)

                # ---- MM1: G = y @ tauK, grouped by lhsT chunk ----
                mm1(ga, 0, 0, 0, FA, 0, start=True)
                mm1(ga, 0, 1, FOLD, FOLD + FA, 0, start=True)
                mm1(ga, 1, 0, 0, FA, 0)
                mm1(ga, 1, 1, FOLD, FOLD + FA, 0)
                mm1(ga, 2, 0, 0, FA, 0, stop=True)
                mm1(ga, 2, 1, FOLD, FOLD + FA, 0, stop=True)
                mm1(gb, 0, 0, FA, FOLD, 0, start=True)
                mm1(gb, 0, 1, FOLD + FA, F, 0, start=True)
                mm1(gb, 1, 0, FA, FOLD, 0)
                mm1(gb, 1, 1, FOLD + FA, F, 0)
                mm1(gb, 2, 0, FA, FOLD, 0, stop=True)
                mm1(gb, 2, 1, FOLD + FA, F, 0, stop=True)
                if not last:
                    # h-row of MM2 fills the gap while DVE starts the u update
                    pe(nc.tensor.matmul(P[0:64, :], ONES[0:1, 0:64],
                                        CB[0:1, C_SH:C_SH + YR],
                                        start=True, stop=False,
                                        skip_group_check=True))

                # ---- u update: TMP = -G + PRES; U = clip(TMP) ----
                def sub_clamp(c):
                    dve(nc.vector.scalar_tensor_tensor(
                        TMP[:, :], ga[:, 128 * c:128 * c + 128], -1.0,
                        PRES[c][:, :], op.mult, op.add))
                    dve(nc.vector.tensor_scalar(
                        UA[c][:, :], TMP[:, :], 0.0, 1.0, op.max, op.min))
                    pool(nc.gpsimd.tensor_sub(PRES[c][:, :], UA[c][:, :],
                                              TQF[:, 128 * c:128 * c + 128]))

                def tr(c):
                    pe(nc.tensor.transpose(TP[c][:, :], UA[c][:, :],
                                           IDENTR[:, :]))

                def cp(c):
                    act(nc.scalar.copy(UFM[c][:, :], TP[c][:, :]))

                sub_clamp(0)
                sub_clamp(1)
                sub_clamp(2)
                sub_clamp(3)
                dve(nc.vector.scalar_tensor_tensor(
                    TMPB[:, :], gb[:, :], -1.0, PRESB[:, :],
                    op.mult, op.add))
                dve(nc.vector.tensor_scalar(
                    UB[:, :], TMPB[:, :], 0.0, 1.0, op.max, op.min))
                pool(nc.gpsimd.tensor_sub(PRESB[:, :], UB[:, :],
                                          TQF[:, FA:FOLD]))

                if last:
                    break   # u_300 is the output; MM2/y of iter 299 are dead

                tr(0)
                cp(0)
                tr(1)
                cp(1)
                mm2(0)
                mm2(1)
                tr(2)
                cp(2)
                mm2(2)
                mm2(3)
                tr(3)
                cp(3)
                mm2(4)
                mm2(5)
                pe(nc.tensor.transpose(TXX[0:FB, 0:128], UB[:, :],
                                       IDENTR[:, :]))          # t4
                act(nc.scalar.copy(UFMB[0:FB, 0:128], TXX[0:FB, 0:128]))
                sp2 = nc.scalar.copy(SCRA[0:32, 4:8], SCRA[0:32, 0:4])
                add_dep_helper(sp2.ins, prev["act"].ins, True, "actsync")
                chain("act", sp2)
                mm2(6)
                mm2(7)
                mm2(8)
                mm2(9, stop=True)

                # ---- y update, chunked so ytrans starts early ----
                for ci in range(3):
                    rw = RW[ci]
                    c0, c1 = 128 * ci, 128 * ci + rw
                    dve(nc.vector.scalar_tensor_tensor(
                        T3[:, c0:c1], P[0:64, c0:c1], 2.0, YP[:, c0:c1],
                        op.mult, op.add))
                    dve(nc.vector.tensor_scalar_max(YBM[:, c0:c1],
                                                    T3[:, c0:c1], 0.0))
                    pe(nc.tensor.transpose(
                        TXX[0:rw, 128 + 64 * ci:128 + 64 * ci + 64],
                        YBM[:, c0:c1], IDENTR[0:64, 0:64]))
                dve(nc.vector.scalar_tensor_tensor(
                    YP[:, :], P[0:64, :], -1.0, YBM[:, :], op.mult, op.add))
                act(nc.scalar.copy(YFM[0:128, 0:192], TXX[0:128, 128:320]))

            zdma1 = nc.sync.dma_start(d_z[:, 0:78], UA[3][64:128, 50:128])
            zdma2 = nc.sync.dma_start(d_z[:, 78:N], UB[64:128, 0:FB])
            for d in (dma_cb, dma_cf, prev["pool"], prev["act"], prev["pe"],
                      prev["dve"], zdma1, zdma2):
                nn = nc.sync.nop()
                add_dep_helper(nn.ins, d.ins, True, "tail fence")
    return nc


LAST_RESULT = None


def kernel(weights, A, A_pos, b, _trace=False):
    weights = np.asarray(weights, np.float32)
    A = np.asarray(A, np.float32)
    A_pos = np.asarray(A_pos, np.float32)
    b = np.asarray(b, np.float32)

    K, h, tau = _build_constants(A, A_pos, b)
    cb = _host_consts(K, h, tau)
    yp0 = np.broadcast_to(tau * h, (64, YR)).astype(np.float32).copy()

    nc = _build_bass()

    in_maps = []
    for core in range(N_CORES):
        w_shard = weights[core * BS:(core + 1) * BS].reshape(BS, N)
        tq = _per_core_tq(w_shard, tau)
        in_maps.append({"cb": cb, "tq": tq, "cf": yp0})

    from concourse.bass_utils import run_bass_kernel_spmd
    res = run_bass_kernel_spmd(nc, in_maps, core_ids=list(range(N_CORES)),
                               trace=_trace)
    global LAST_RESULT
    LAST_RESULT = res
    outs = [np.asarray(res.results[c]["z"]) for c in range(N_CORES)]
    z = np.concatenate(outs, axis=0).reshape(B, N_GRID, N_GRID)
    return z.astype(np.float32)


if __name__ == "__main__":
    _build_bass()
    print("bass build OK")
